# revision 70
# baseline (speedup 1.0000x reference)
"""Causal GQA self-attention (B=1, T=2048, C=2048, 32 heads / 8 KV groups,
head_size 64, partial RoPE 16) on 8 Trainium2 NeuronCores.

Sharding: tensor-parallel over the 8 query groups. Core g computes its
group's qkv projection (x @ W_attn_g.T, feature-major), RoPE, causal
attention for its 4 heads, then an AllToAll redistributes the attention
output so core g holds y[:, t_slice_g] for ALL heads; each core then
computes out[t_slice_g, :] = y_slice @ W_proj.T + b_proj.

v4 structure (fully software-pipelined, in-order PE fill):
- RoPE rotate-half via DVE stream_shuffle (partition permutation within
  32-groups) instead of SBUF->SBUF DMAs; sin-multiply + k-duplication on
  the (otherwise idle) Pool engine.
- yts tiles carry 65 rows: row 64 is the sumexp row straight from the
  ones-column of the PV matmul, so no single-partition staging copies.
- projection matmuls are interleaved between attention kb-iterations so
  the in-order PE fills softmax-latency gaps: phase 1 interleaves chunk
  c's qkv matmuls with pair-0 attention of chunk c-1; phase 2
  interleaves the pair-0 half of the output projection with pair-1
  attention.
- softmax exp split: ScalarE exact exp for full blocks + diag m=0; DVE
  Schraudolph exp2 bit-trick for diag m>=1 with the causal mask folded
  into the bias tile.
- sumexp reciprocals bounce through DRAM to a [128, 32] layout so the
  reciprocal runs wide, then broadcast back with a replicating read AP.
"""
import os
import sys
from contextlib import ExitStack

sys.path.insert(0, "/opt/trn_rl_repo")

import numpy as np
import concourse.bass as bass
import concourse.mybir as mybir
import concourse.tile as tile
from concourse import bacc
from concourse.bass_utils import run_bass_kernel_spmd
from concourse.masks import make_identity

F32 = mybir.dt.float32
F16 = mybir.dt.float16
I16 = mybir.dt.int16
EXP = mybir.ActivationFunctionType.Exp
IDENT = mybir.ActivationFunctionType.Identity

NCORES = 8
T = 2048
C = 2048
HS = 64
QG = 384          # per-group qkv features: 4 q heads + k + v
TS = T // NCORES  # 256, per-core output T slice
SCALE = 0.125     # 1/sqrt(64)
NKB = T // 128    # 16 key blocks
NCH = T // 512    # 4 qt chunks

# Schraudolph exp2 trick (fp16): exp(s*SCALE) ~= bitcast(int16(s*AH + BH))
AH = 1024.0 * 1.4426950409 * SCALE
BH = 15360.0 - 44.0
# masked-lane bias fill: for |score| <= 88, sc*AH + MASKF stays inside
# (-65504, -32768) -> finite fp16, int16-saturates -> -0.0
MASKF = -49152.0
DVE_MIN_M = int(os.environ.get("K_DVE_MIN_M", "1"))  # m >= this -> DVE trick

# rope rotate-half: within each 32-partition group, rows 0:8 <-> 8:16,
# rows 16:32 identity (q heads sit at partition 0 and 64 of each tile)
SHUF = list(range(8, 16)) + list(range(0, 8)) + list(range(16, 32))

_nc_cache = {}


def _build(for_sim=False, reps=1):
    nc = bacc.Bacc("TRN2", target_bir_lowering=False, debug=False,
                   num_devices=NCORES)
    xT = nc.declare_dram_parameter("xT", [C, T], F16, isOutput=False)
    wqkvT = nc.declare_dram_parameter("wqkvT", [C, QG], F16, isOutput=False)
    bqkv = nc.declare_dram_parameter("bqkv", [QG, 1], F32, isOutput=False)
    ropeCS = nc.declare_dram_parameter("ropeCS", [32, T], F16, isOutput=False)
    wprojT = nc.declare_dram_parameter("wprojT", [C, C], F16, isOutput=False)
    bproj = nc.declare_dram_parameter("bproj", [1, C], F32, isOutput=False)
    out = nc.declare_dram_parameter("out", [TS, C], F16, isOutput=True)

    with tile.TileContext(nc) as tc:
      with (
          tc.tile_pool(name="persist", bufs=1) as persist,
          tc.tile_pool(name="dram", bufs=1, space="DRAM") as dram,
      ):
            # q0|q1, q2|q3, k|v  — feature-major [feat, T]
            qkv0 = persist.tile([128, T], F16)
            qkv1 = persist.tile([128, T], F16)
            qkv2 = persist.tile([128, T], F16)
            kdup = persist.tile([128, T], F16)       # rows 64:128 = k copy
            v_sb = persist.tile([128, NKB, 65], F16)  # v t-major + ones col
            rtmp = persist.tile([128, T], F16)       # rope rotate staging
            yts = [persist.tile([65, T], F16, name=f"yts{i}", tag=f"yts{i}")
                   for i in range(4)]                # row 64 = sumexp
            bprojb = persist.tile([128, C], F32)
            identf = persist.tile([128, 128], F32)
            ident = persist.tile([128, 128], F16)
            maskb = persist.tile([128, 512], F32)    # exp2-trick bias+mask
            mask01f = persist.tile([128, 128], F32)
            mask01 = persist.tile([128, 128], F16)   # 1 if c >= p else 0
            ropeC_sb = persist.tile([128, T], F16)
            ropeS_sb = persist.tile([128, T], F16)

            # PE warmup source: memset FIRST so the warmup matmuls issue
            # with ~no dependency latency
            wrm = persist.tile([128, 512], F16)
            nc.vector.memset(wrm[:], 0.0)

            make_identity(nc, identf[:])
            nc.vector.tensor_copy(ident[:], identf[:])
            nc.gpsimd.memset(mask01f[:], 1.0)
            nc.gpsimd.affine_select(
                out=mask01f[:], in_=mask01f[:],
                compare_op=mybir.AluOpType.is_ge, fill=0.0,
                base=0, pattern=[[1, 128]], channel_multiplier=-1,
            )
            nc.vector.tensor_copy(mask01[:], mask01f[:])
            # maskb[p, c] = BH if c >= p else MASKF; sliced [0:w] so the
            # triangular 128 cols land on the diagonal sub-block.
            nc.gpsimd.memset(maskb[:], BH)
            nc.gpsimd.affine_select(
                out=maskb[:, 0:128], in_=maskb[:, 0:128],
                compare_op=mybir.AluOpType.is_ge, fill=MASKF,
                base=0, pattern=[[1, 128]], channel_multiplier=-1,
            )
            nc.vector.memset(v_sb[:, :, 64:65], 1.0)
            # rope tables: C rows default 1.0, S rows default 0.0; the 16
            # rope rows land at 0:16 and 64:80 (both q head positions).
            # Slab DMAs are emitted after the startup-critical wq/x loads.
            nc.vector.memset(ropeC_sb[:], 1.0)
            nc.vector.memset(ropeS_sb[:], 0.0)
            with (
                tc.tile_pool(name="probs", bufs=6) as probsp,
                tc.tile_pool(name="trick", bufs=4) as trickp,
                tc.tile_pool(name="small", bufs=1) as smallp,
                tc.tile_pool(name="wp", bufs=6) as wpp,
                tc.tile_pool(name="ymy", bufs=1) as ymyp,
                tc.tile_pool(name="osb", bufs=2) as osbp,
                tc.tile_pool(name="part", bufs=1) as partp,
            ):
              for _rep in range(reps):
                # pair-0 projection partials (+bias), built during pair-1
                opart = partp.tile([128, 8, 512], F32)
                wpin = wprojT.ap().rearrange("(g two p) j -> two p g j",
                                             two=2, p=128)
                # rows 0:128 = raw (unnormalized) y for the pair's 2 heads,
                # rows 128:130 = reciprocal sumexp for head 0 / head 1
                # (one AllToAll per pair — collective launches cost ~9us on
                # HW, so the sums ride the same collective)
                y_send = [dram.tile([NCORES, 130, TS], F16, name=f"ysend{p}",
                                    tag=f"ysend{p}") for p in range(2)]
                # y_my free layout per g: [pair=2, t=256] -> (g*512+pair*256+t)
                y_my = ymyp.tile([128, 16 * TS], F16)
                ymyv = y_my[:].rearrange("p (g t2) -> p g t2", g=NCORES)
                y_recv = [dram.tile([NCORES, 130, TS], F16, name=f"yrecv{p}",
                                    tag=f"yrecv{p}") for p in range(2)]
                ps_stack = ExitStack()
                psc = ps_stack.enter_context(
                    tc.tile_pool(name="psc", bufs=2, space="PSUM"))
                psy = ps_stack.enter_context(
                    tc.tile_pool(name="psy", bufs=1, space="PSUM"))

                lhs_base = (qkv2[0:64], kdup[64:128])
                rshs = [smallp.tile([128, 32], F16, name=f"rsh{p}",
                                    tag=f"rsh{p}") for p in range(2)]

                def attn_iters(pair, j):
                    """Generator: one yield per kb0 iteration of causal
                    attention for a head pair on query chunk j. Emits the
                    interleaved score matmuls, split exp, y matmuls; on
                    completion copies yps (incl sumexp row 64) to yts."""
                    qt_tile = (qkv0, qkv1)[pair]
                    tsl = slice(j * 512, j * 512 + 512)
                    nkb_j = 4 * j + 4
                    yps = [psy.tile([65, 512], F32, name=f"y{h}", tag=f"y{h}")
                           for h in range(2)]
                    for kb0 in range(0, nkb_j, 2):
                        diag = kb0 >= 4 * j
                        scs = [psc.tile([128, 1024], F32, name="sc",
                                        tag="sc") for h in range(2)]
                        probs = [probsp.tile([128, 1024], F16,
                                             name="pr", tag="pr")
                                 for h in range(2)]
                        for i, kb in ((0, kb0), (1, kb0 + 1)):
                            rag = max(0, kb - 4 * j) * 128
                            for h in range(2):
                                nc.tensor.matmul(
                                    scs[h][:, i * 512 + rag:i * 512 + 512],
                                    lhs_base[h][:, kb * 128:kb * 128 + 128],
                                    qt_tile[64 * h:64 * h + 64,
                                            j * 512 + rag:j * 512 + 512],
                                    tile_position=(64 * h, 0),
                                )
                        if not diag:
                            for h in range(2):
                                nc.scalar.activation(
                                    probs[h][:], scs[h][:], EXP, scale=SCALE)
                        else:
                            for h in range(2):
                                for i, kb in ((0, kb0), (1, kb0 + 1)):
                                    m = kb - 4 * j
                                    rag = m * 128
                                    w = 512 - rag
                                    po = i * 512 + rag
                                    if m < DVE_MIN_M:
                                        nc.scalar.activation(
                                            probs[h][:, po:po + w],
                                            scs[h][:, po:po + w],
                                            EXP, scale=SCALE)
                                        # zero probs above the diagonal
                                        nc.vector.tensor_mul(
                                            probs[h][:, po:po + 128],
                                            probs[h][:, po:po + 128],
                                            mask01[:])
                                    else:
                                        th = trickp.tile(
                                            [128, 512], F16,
                                            name="th", tag="th")
                                        nc.vector.scalar_tensor_tensor(
                                            out=th[:, 0:w],
                                            in0=scs[h][:, po:po + w],
                                            scalar=AH,
                                            in1=maskb[:, 0:w],
                                            op0=mybir.AluOpType.mult,
                                            op1=mybir.AluOpType.add,
                                        )
                                        nc.vector.tensor_copy(
                                            probs[h][:, po:po + w].bitcast(I16),
                                            th[:, 0:w])
                        # filler lands here too, covering the exp latency
                        # between the score and PV matmuls
                        yield
                        for i, kb in ((0, kb0), (1, kb0 + 1)):
                            rag = max(0, kb - 4 * j) * 128
                            for h in range(2):
                                nc.tensor.matmul(
                                    yps[h][:, rag:512],
                                    v_sb[:, kb, :],
                                    probs[h][:, i * 512 + rag:i * 512 + 512],
                                    start=(kb == 0), stop=(kb == nkb_j - 1),
                                )
                        yield
                    for h in range(2):
                        hd = pair * 2 + h
                        # rows 0:64 = y, row 64 = sumexp — one evac copy
                        nc.vector.tensor_copy(yts[hd][:, tsl], yps[h][:])
                        # sumexp slab for the wide per-pair reciprocal
                        (nc.scalar, nc.sync)[h].dma_start(
                            rshs[pair][h * 64 + j * 16:h * 64 + j * 16 + 16,
                                       :],
                            yts[hd][64:65, tsl])

                def stage_chunk(pair, j, engs=(None, None)):
                    """Stage chunk j's slice of raw y into the pair's send
                    buffer as soon as the chunk completes, so only chunk 3's
                    staging sits on the tail chain."""
                    tsl = slice(j * 512, j * 512 + 512)
                    for h in range(2):
                        hd = pair * 2 + h
                        eng = engs[h] or nc.gpsimd
                        eng.dma_start(
                            y_send[pair][2 * j:2 * j + 2,
                                         h * 64:h * 64 + 64, :].rearrange(
                                "i f t -> f i t"),
                            yts[hd][0:64, tsl].rearrange(
                                "d (i t) -> d i t", i=2),
                        )

                def pair_send(pair):
                    """Chunk 3 just finished: stage its slice + the
                    reciprocal sumexp rows on the fast queues, then one
                    AllToAll. Normalization happens on the receive side."""
                    rsh = rshs[pair]
                    with nc.allow_low_precision(
                            reason="fp16 reciprocal of fp16 sumexp"):
                        nc.vector.reciprocal(rsh[:], rsh[:])
                    stage_chunk(pair, 3, engs=(nc.scalar, nc.sync))
                    for h in range(2):
                        # rsh linear index h*2048 + t matches (g, t) order
                        eng = (nc.scalar, nc.sync)[h]
                        eng.dma_start(
                            y_send[pair][:, 128 + h, :],
                            rsh[h * 64:h * 64 + 64, :],
                        )
                    if for_sim:
                        nc.sync.dma_start(y_recv[pair][:], y_send[pair][:])
                    else:
                        nc.gpsimd.collective_compute(
                            "AllToAll",
                            mybir.AluOpType.bypass,
                            replica_groups=[list(range(NCORES))],
                            ins=[y_send[pair].opt()],
                            outs=[y_recv[pair].opt()],
                        )

                def pair_recv(pair):
                    """Unpack the AllToAll: copy raw y into y_my, broadcast
                    the reciprocal sumexp rows across head dims, normalize.
                    Split by group-halves so the first half of the
                    projection's K blocks can start while the second half is
                    still normalizing."""
                    recb = smallp.tile([128, NCORES, TS], F16, name="recb",
                                       tag="recb")
                    for gh in range(2):
                        gs = slice(gh * 4, gh * 4 + 4)
                        eng0 = (nc.scalar, nc.sync)[gh]
                        eng0.dma_start(
                            ymyv[:, gs, pair * TS:pair * TS + TS],
                            y_recv[pair][gs, 0:128, :].rearrange(
                                "g p t -> p g t"),
                        )
                        for h in range(2):
                            rv = y_recv[pair][gs, 128 + h, :]
                            eng = (nc.sync, nc.scalar)[h]
                            eng.dma_start(
                                recb[h * 64:h * 64 + 64, gs, :],
                                bass.AP(tensor=rv.tensor, offset=rv.offset,
                                        ap=[[0, 64]] + list(rv.ap)),
                            )
                        nc.vector.tensor_mul(
                            ymyv[:, gs, pair * TS:pair * TS + TS],
                            ymyv[:, gs, pair * TS:pair * TS + TS],
                            recb[:, gs, :])

                # ------ Phase 1: projection + pair-0 attention, PE-packed --
                wqp = ExitStack()
                wq_pool = wqp.enter_context(tc.tile_pool(name="wq", bufs=1))
                xtp = wqp.enter_context(tc.tile_pool(name="xt", bufs=4))
                ps1 = wqp.enter_context(
                    tc.tile_pool(name="ps1", bufs=2, space="PSUM"))

                wq_sb = wq_pool.tile([128, 16 * QG], F16)

                def load_wq_cts(ct0, ct1, eng):
                    eng.dma_start(
                        wq_sb[:, ct0 * QG:ct1 * QG].rearrange(
                            "p (ct f) -> p ct f", ct=ct1 - ct0),
                        wqkvT.ap()[ct0 * 128:ct1 * 128, :].rearrange(
                            "(ct p) f -> p ct f", p=128),
                    )

                b_sb = wq_pool.tile([128, 3], F32)
                nc.gpsimd.dma_start(
                    b_sb[:].rearrange("p (i o) -> p i o", i=3),
                    bqkv.ap().rearrange("(i p) o -> p i o", p=128),
                )

                qkv_tiles = [qkv0, qkv1, qkv2]
                xts_cache = {}

                def load_xt_ch(tch, ch, engs=(nc.sync, nc.scalar)):
                    tsl = slice(tch * 512, tch * 512 + 512)
                    xt = xtp.tile([128, 8, 512], F16)
                    # split loads so the first matmuls start sooner
                    for hf in range(2):
                        eng = engs[hf]
                        eng.dma_start(
                            xt[:, hf * 4:hf * 4 + 4, :],
                            xT[ch * 1024 + hf * 512:
                               ch * 1024 + hf * 512 + 512,
                               tsl].rearrange(
                                "(ct p) t -> p ct t", p=128),
                        )
                    xts_cache.setdefault(tch, []).append(xt)

                def load_xt(tch):
                    for ch in range(2):  # 1MB x-tile halves
                        load_xt_ch(tch, ch)

                def proj_block(tch, fi, xts):
                    """qkv projection for feature block fi of chunk tch:
                    16 accumulating matmuls + bias + rope (+ k-dup and
                    v-transpose for fi == 2). Yields between matmuls."""
                    tsl = slice(tch * 512, tch * 512 + 512)
                    pq = ps1.tile([128, 512], F32, name="pq", tag="pq")
                    for ch in range(2):
                        for c8 in range(8):
                            ct = ch * 8 + c8
                            nc.tensor.matmul(
                                pq[:],
                                wq_sb[:, ct * QG + fi * 128:
                                      ct * QG + fi * 128 + 128],
                                xts[ch][:, c8, :],
                                start=(ct == 0), stop=(ct == 15),
                            )
                            yield
                    if _rep == 0 and tch == 0 and fi == 2:
                        # rope table slabs: emitted here so their DMAs queue
                        # behind the startup-critical wq/x loads
                        for base in (0, 64):
                            nc.gpsimd.dma_start(ropeC_sb[base:base + 16, :],
                                                ropeCS[0:16, :])
                            nc.gpsimd.dma_start(ropeS_sb[base:base + 16, :],
                                                ropeCS[16:32, :])
                    ti = qkv_tiles[fi]
                    nc.scalar.activation(
                        ti[:, tsl], pq[:], IDENT,
                        bias=b_sb[:, fi:fi + 1],
                    )
                    # rope: rotate-half via stream_shuffle (partition perm
                    # within 32-groups), sin-mul on Pool, cos-mul + add on
                    # DVE. Non-rope rows: S=0 zeroes the rotated garbage,
                    # C=1 keeps the original values.
                    if fi != 2:
                        nc.vector.stream_shuffle(rtmp[:, tsl], ti[:, tsl],
                                                 SHUF)
                        nc.gpsimd.tensor_mul(rtmp[:, tsl], rtmp[:, tsl],
                                             ropeS_sb[:, tsl])
                        nc.vector.tensor_mul(ti[:, tsl], ti[:, tsl],
                                             ropeC_sb[:, tsl])
                        nc.vector.tensor_add(ti[:, tsl], ti[:, tsl],
                                             rtmp[:, tsl])
                    else:
                        # k rope touches rows 0:16 only (v at 64:128)
                        nc.vector.stream_shuffle(rtmp[0:32, tsl],
                                                 ti[0:32, tsl], SHUF)
                        nc.gpsimd.tensor_mul(rtmp[0:16, tsl], rtmp[0:16, tsl],
                                             ropeS_sb[0:16, tsl])
                        nc.vector.tensor_mul(ti[0:16, tsl], ti[0:16, tsl],
                                             ropeC_sb[0:16, tsl])
                        nc.vector.tensor_add(ti[0:16, tsl], ti[0:16, tsl],
                                             rtmp[0:16, tsl])
                        # k dup to partitions 64:128 (odd heads) on Pool and
                        # v transpose into t-major blocks on PE
                        nc.gpsimd.tensor_copy(kdup[64:128, tsl],
                                              qkv2[0:64, tsl])
                        tp = psc.tile([128, 4, 64], F16, name="tp", tag="sc")
                        for q in range(4):
                            kb = tch * 4 + q
                            nc.tensor.matmul(
                                tp[:, q, :],
                                qkv2[64:128, kb * 128:kb * 128 + 128],
                                ident[64:128, 64:128],
                                is_transpose=True,
                                start=(q == 0), stop=(q == 3),
                            )
                        yield
                        nc.vector.tensor_copy(
                            v_sb[:, tch * 4:tch * 4 + 4, 0:64], tp[:])

                def proj_chunk(tch):
                    xts = xts_cache[tch]
                    for fi in (2, 0, 1):
                        yield from proj_block(tch, fi, xts)
                        if fi == 2 and tch + 1 < NCH:
                            load_xt(tch + 1)  # prefetch next chunk's x

                def interleave(work, filler, ratio):
                    """Pull `ratio` filler items after each work item."""
                    for _ in work:
                        for _ in range(ratio):
                            if next(filler, "done") == "done":
                                pass
                    return

                # chunk 0 projection runs un-interleaved (nothing to fill).
                # Startup DMA order: alternate wq parts and x pieces across
                # the two HW queues so the first matmuls start ~2us in.
                # PE warmup: dependency-free matmuls on a zeroed tile ramp
                # the clock out of its low pstate and cover the first wq/x
                # DMA latency (first rep only — later reps overlap the
                # previous rep's tail)
                if _rep == 0:
                    pqw = ps1.tile([128, 512], F32, name="pqw", tag="pq")
                    for i in range(8):
                        nc.tensor.matmul(pqw[:], wrm[:, 0:128], wrm[:],
                                         start=(i == 0), stop=(i == 7))
                # startup: alternate wq and x pieces across the two HW
                # queues in consumption order (sync first — the scalar/ACT
                # queue is delayed by the activation-table preamble)
                load_wq_cts(0, 1, nc.sync)      # first matmul's weights
                load_xt_ch(0, 0, (nc.scalar, nc.sync))
                load_wq_cts(1, 4, nc.scalar)
                load_wq_cts(4, 8, nc.sync)
                load_xt_ch(0, 1, (nc.scalar, nc.scalar))
                load_wq_cts(8, 12, nc.sync)
                load_wq_cts(12, 16, nc.sync)
                for _ in proj_chunk(0):
                    pass
                # chunks 1..3 interleave with pair-0 attention of c-1
                for tch in range(1, NCH):
                    work = attn_iters(0, tch - 1)   # 2 yields per iteration
                    filler = proj_chunk(tch)        # ~49 yields
                    niter = 4 * tch
                    ratio = (49 + niter) // (niter + 1)
                    # lead filler: let the xt DMAs + first matmuls go first
                    for _ in range(ratio):
                        next(filler, None)
                    interleave(work, filler, ratio)
                    for _ in filler:
                        pass
                    stage_chunk(0, tch - 1)
                wqp.close()

                # ---------- pair-1 attention + overlapped projection -------
                # bprojb rides the (backlogged) scalar queue so its 1MB
                # broadcast never jumps ahead of the startup loads
                if _rep == 0:
                    bp = bproj[0, :]
                    nc.scalar.dma_start(
                        bprojb[:],
                        bass.AP(tensor=bp.tensor, offset=bp.offset,
                                ap=[[0, 128]] + list(bp.ap)),
                    )
                psp0 = ps_stack.enter_context(
                    tc.tile_pool(name="psp0", bufs=2, space="PSUM"))

                def proj_par0(j):
                    # pair-0 half of out[:, jsl]: runs under pair-1
                    jsl = slice(j * 512, j * 512 + 512)
                    wp0 = wp0s[j]
                    pp = [psp0.tile([128, 512], F32, name="pp", tag="pp")
                          for tt in range(2)]
                    for g8 in range(8):
                        off = g8 * 512
                        for tt in range(2):
                            nc.tensor.matmul(
                                pp[tt][:],
                                y_my[:, off + tt * 128:off + tt * 128 + 128],
                                wp0[:, g8, :],
                                start=(g8 == 0), stop=(g8 == 7),
                            )
                        yield
                    for tt in range(2):
                        nc.vector.tensor_add(
                            opart[:, j * 2 + tt, :], pp[tt][:],
                            bprojb[:, jsl])

                wp0s = {}
                # attn(0, 3) has no projection work left to hide behind;
                # prefetch all four W_proj pair-0 slices during it instead.
                for j in range(NCH):
                    wp0s[j] = wpp.tile([128, 8, 512], F16, name="wp",
                                       tag="wp")
                    nc.sync.dma_start(
                        wp0s[j][:],
                        wpin[0:1, :, :, j * 512:j * 512 + 512])
                for _ in attn_iters(0, 3):
                    pass
                pair_send(0)

                def load_wp1(jc):
                    wp1s[jc] = wpp.tile([128, 8, 512], F16, name="wp",
                                        tag="wp")
                    nc.sync.dma_start(
                        wp1s[jc][:],
                        wpin[1:2, :, :, jc * 512:jc * 512 + 512])

                wp1s = {}
                for j in range(NCH):
                    work = attn_iters(1, j)
                    filler = proj_par0(j - 1) if j > 0 else iter(())
                    niter = 4 * j + 4
                    ratio = max(1, (8 + niter) // niter)
                    interleave(work, filler, ratio)
                    for _ in filler:
                        pass
                    if j < 3:
                        stage_chunk(1, j)
                    if j == 0:
                        # pair-0 receive/normalize lands here so its DVE ops
                        # hide under chunk-0 attention, right before pp(0)
                        pair_recv(0)
                    if j == 1:
                        # prefetch phase-3 W_proj slices while the DMA
                        # device is quiet — they must not land on the
                        # pair-1 finish chain
                        load_wp1(0)
                        load_wp1(1)
                    elif j == 2:
                        load_wp1(2)
                pair_send(1)
                for _ in proj_par0(3):  # fills the PE gap while a2a drains
                    pass
                pair_recv(1)
                ps_stack.close()

                # ------- Phase 3 tail: pair-1 half + staged partials -------
                with tc.tile_pool(name="psp", bufs=4, space="PSUM") as psp:
                    for jc in range(4):
                        jsl = slice(jc * 512, jc * 512 + 512)
                        if jc == 0:
                            load_wp1(3)
                        wp1 = wp1s[jc]
                        pp = [psp.tile([128, 512], F32, name="pp1", tag="pp1")
                              for tt in range(2)]
                        for g8 in range(8):
                            off = g8 * 512 + 256
                            for tt in range(2):
                                nc.tensor.matmul(
                                    pp[tt][:],
                                    y_my[:, off + tt * 128:off + tt * 128 + 128],
                                    wp1[:, g8, :],
                                    start=(g8 == 0), stop=(g8 == 7),
                                )
                        for tt in range(2):
                            osbt = osbp.tile([128, 512], F16, name="osbt",
                                             tag="osbt")
                            nc.vector.tensor_add(osbt[:], pp[tt][:],
                                                 opart[:, jc * 2 + tt, :])
                            nc.sync.dma_start(
                                out[tt * 128:tt * 128 + 128, jsl], osbt[:])

    nc.finalize()
    return nc


def _get_nc():
    if "nc" not in _nc_cache:
        _nc_cache["nc"] = _build()
    return _nc_cache["nc"]


def _prepare_in_maps(x, cos, sin, W_attn, b_attn, W_proj, b_proj):
    x = np.asarray(x, dtype=np.float32)
    cos = np.asarray(cos, dtype=np.float32)
    sin = np.asarray(sin, dtype=np.float32)
    W_attn = np.asarray(W_attn, dtype=np.float32)
    b_attn = np.asarray(b_attn, dtype=np.float32)
    W_proj = np.asarray(W_proj, dtype=np.float32)
    b_proj = np.asarray(b_proj, dtype=np.float32)

    big = np.float16
    xT = np.ascontiguousarray(x[0].T).astype(big)          # [C, T]
    wprojT = np.ascontiguousarray(W_proj.T).astype(big)    # [C(in f), C(out j)]
    bproj = b_proj.reshape(1, C)

    ct, st = cos.T.astype(np.float32), sin.T.astype(np.float32)  # [16, T]
    ropeCS = np.empty((32, T), np.float32)
    ropeCS[0:16] = ct
    ropeCS[16:24] = -st[0:8]
    ropeCS[24:32] = st[8:16]

    in_maps = []
    for g in range(NCORES):
        wg = np.ascontiguousarray(W_attn[g * QG:(g + 1) * QG].T).astype(big)
        bg = np.ascontiguousarray(b_attn[g * QG:(g + 1) * QG].reshape(QG, 1))
        in_maps.append({
            "xT": xT, "wqkvT": wg, "bqkv": bg,
            "ropeCS": ropeCS.astype(big),
            "wprojT": wprojT, "bproj": bproj,
        })
    return in_maps


def kernel(x, cos, sin, W_attn, b_attn, W_proj, b_proj):
    nc = _get_nc()
    in_maps = _prepare_in_maps(x, cos, sin, W_attn, b_attn, W_proj, b_proj)
    res = run_bass_kernel_spmd(nc, in_maps, list(range(NCORES)))
    out = np.concatenate([res.results[g]["out"] for g in range(NCORES)], axis=0)
    return out.reshape(1, T, C).astype(np.float32)
